# revision 7
# baseline (speedup 1.0000x reference)
"""MoE router (top-2, capacity=768) on 8 Trainium2 NeuronCores.

Strategy
--------
Data-parallel over tokens: core m owns tokens [m*4096, (m+1)*4096) of the
flattened [32768, 1024] input.

Phase 1 (device, DMA-bound): router logits via fp16 hi/lo-split matmuls
(exact fp32-quality: x = x_hi + x_lo, W' = 1024*W = W_hi + W_lo, keep the
three significant cross terms), softmax (ACT exp with accumulated sum),
top-2 via the DVE max8/max_index instructions.  Emits router_probs and a
small per-token meta record (idx0, idx1, p0, p1, pn0, pn1).

Host exchange (tiny): exact global sequential capacity assignment in flat
(token, k) order from the 32768x2 index table -- the inherently sequential
cross-shard part -- then per-slot target codes f = 2*e + k (or -1 if the
slot is over capacity).

Phase 2 (device, write-bound): builds dispatch/combine [tokens, 64, 2] by
comparing an iota row 0..127 against the per-token codes; combine is
dispatch scaled by the renormalized top-2 probs.
"""

import os
import time

import numpy as np

import concourse.bass as bass
import concourse.mybir as mybir
import bass_rust
from concourse.tile import TileContext
from concourse.vector_clock import ScopedClock
from concourse.bass_utils import run_bass_kernel_spmd


class SplitDrainTileContext(TileContext):
    """TileContext that caps sem-waits at one per instruction -- the
    walrus build in this container rejects any instruction carrying more
    than one sync wait ('Too many sync wait commands').  Excess waits are
    hoisted onto same-engine NoOps emitted just before the instruction,
    which is semantically identical (the engine queue is FIFO)."""

    MAX_WAITS = 1

    def _hoist_excess_waits(self, inst):
        si = inst.sync_info
        if si is None or not si.on_wait or len(si.on_wait) <= self.MAX_WAITS:
            return
        waits = list(si.on_wait)
        keep, extra = waits[: self.MAX_WAITS], waits[self.MAX_WAITS:]
        for w in extra:
            nop = mybir.InstNoOp(
                name=self.nc.get_next_instruction_name(), ins=[], outs=[],
                engine=inst.engine)
            nop.sync_info = bass_rust.SyncInfo(on_wait=[w], on_update=[])
            self._add_instruction(nop)
        si.on_wait = keep

    def _commit_instruction(self, inst, lazy_reg_writes=True):
        if inst.engine != mybir.EngineType.Unassigned:
            self._hoist_excess_waits(inst)
        return super()._commit_instruction(inst, lazy_reg_writes)

    def _drain_and_barrier(self, tick_clock, wait_clock):
        ticks = list(tick_clock.global_clock)
        n = len(ticks)
        for i, t in enumerate(ticks):
            if t:
                d = self.nc.sync.drain()
                clk = bass_rust.VectorClock(
                    [t if j == i else 0 for j in range(n)])
                wait_clock.add_sem_waits(d.ins, ScopedClock({None: clk}))
        self.nc.all_engine_barrier()
        assert self.sems is not None
        popped = self.nc._tile_sem_poison_stack.pop()
        assert popped is self._sem_poison
        self.nc.clear_and_free_semaphores(list(self.sems.allocated().values()))
        self.nc.all_engine_barrier()

F32 = mybir.dt.float32
F16 = mybir.dt.float16
U32 = mybir.dt.uint32

N_CORES = 8
B, S, D, E, K = 4, 8192, 1024, 64, 2
N = B * S                    # 32768 tokens
NT = N // N_CORES            # 4096 tokens per core
CAPACITY = 768
W_SCALE = 1024.0             # router weight pre-scale (undone inside exp)
TILE = 128                   # tokens per tile (partition dim)
GROUP = 512                  # tokens per matmul group (PSUM free dim)
NCHUNK = D // 128            # contraction chunks


def build_phase1(n_tok=NT):
    """logits + softmax + top2.  Inputs are pre-transposed fp16 hi/lo."""
    ngroup = n_tok // GROUP
    nc = bass.Bass()
    xhiT = nc.dram_tensor("xhiT", [D, n_tok], F16, kind="ExternalInput")
    xloT = nc.dram_tensor("xloT", [D, n_tok], F16, kind="ExternalInput")
    # whl[:, c*128 : c*128+64]  = W_hi[:, c*128:(c+1)*128].T   (fp16)
    # whl[:, c*128+64 : c*128+128] = W_lo chunk transposed
    whl = nc.dram_tensor("whl", [128, D], F16, kind="ExternalInput")
    # whl2[:, c*128+64 : c*128+128] = W_hi chunk transposed, zeros elsewhere
    whl2 = nc.dram_tensor("whl2", [128, D], F16, kind="ExternalInput")
    ident = nc.dram_tensor("ident", [128, 128], F32, kind="ExternalInput")
    probs_o = nc.dram_tensor("probs", [n_tok, E], F32, kind="ExternalOutput")
    meta_o = nc.dram_tensor("meta", [n_tok, 8], F32, kind="ExternalOutput")

    xhiT_r = xhiT[:, :].rearrange("(c p) (g t) -> g p c t", p=128, t=GROUP)
    xloT_r = xloT[:, :].rearrange("(c p) (g t) -> g p c t", p=128, t=GROUP)

    with SplitDrainTileContext(nc) as tc:
        with (
            tc.tile_pool(name="const", bufs=1) as cpool,
            tc.tile_pool(name="xin", bufs=2) as xpool,
            tc.tile_pool(name="work", bufs=3) as wpool,
            tc.tile_pool(name="small", bufs=4) as spool,
            tc.tile_pool(name="ps_mm", bufs=2, space="PSUM") as pmm,
            tc.tile_pool(name="ps_tr", bufs=2, space="PSUM") as ptr,
        ):
            w_sb = cpool.tile([128, D], F16)
            nc.sync.dma_start(w_sb, whl[:, :])
            w2_sb = cpool.tile([128, D], F16)
            nc.sync.dma_start(w2_sb, whl2[:, :])
            id_sb = cpool.tile([128, 128], F32)
            nc.sync.dma_start(id_sb, ident[:, :])

            for g in range(ngroup):
                xh = xpool.tile([128, NCHUNK, GROUP], F16, tag="xh")
                nc.sync.dma_start(xh, xhiT_r[g])
                xl = xpool.tile([128, NCHUNK, GROUP], F16, tag="xl")
                nc.sync.dma_start(xl, xloT_r[g])

                # logits^T accumulation: rows 0:64 = x_hi*W_hi (+ x_lo*W_hi),
                # rows 64:128 = x_hi*W_lo.  Sum after.
                ps = pmm.tile([128, GROUP], F32)
                for c in range(NCHUNK):
                    nc.tensor.matmul(
                        ps, lhsT=w_sb[:, c * 128:(c + 1) * 128],
                        rhs=xh[:, c, :], start=(c == 0), stop=False,
                    )
                for c in range(NCHUNK):
                    nc.tensor.matmul(
                        ps, lhsT=w2_sb[:, c * 128:(c + 1) * 128],
                        rhs=xl[:, c, :], start=False, stop=(c == NCHUNK - 1),
                    )
                lo_sb = wpool.tile([64, GROUP], F32, tag="lo_sb")
                nc.scalar.copy(lo_sb, ps[64:128, :])
                logT = wpool.tile([64, GROUP], F32, tag="logT")
                nc.vector.tensor_add(logT, ps[0:64, :], lo_sb)

                # transpose back to token-major [128, 64] tiles
                pT = ptr.tile([128, 4 * E], F32)
                for j in range(4):
                    nc.tensor.transpose(
                        pT[:, j * E:(j + 1) * E],
                        logT[:, j * 128:(j + 1) * 128],
                        id_sb[0:64, 0:64],
                    )

                for j in range(4):
                    i = g * 4 + j          # global tile index
                    row0 = i * TILE
                    ex = wpool.tile([128, E], F32, tag="ex")
                    sumex = spool.tile([128, 1], F32, tag="sumex")
                    nc.scalar.activation(
                        ex, pT[:, j * E:(j + 1) * E],
                        mybir.ActivationFunctionType.Exp,
                        scale=1.0 / W_SCALE, accum_out=sumex,
                    )
                    rs = spool.tile([128, 1], F32, tag="rs")
                    nc.vector.reciprocal(rs, sumex)
                    pout = wpool.tile([128, E], F32, tag="pout")
                    nc.scalar.mul(pout, ex, rs)
                    nc.sync.dma_start(probs_o[row0:row0 + TILE, :], pout)

                    m8 = spool.tile([128, 8], F32, tag="m8")
                    i8 = spool.tile([128, 8], U32, tag="i8")
                    nc.vector.max(m8, ex)
                    nc.vector.max_index(i8, m8, ex)

                    meta = spool.tile([128, 8], F32, tag="meta")
                    nc.vector.tensor_copy(meta[:, 0:2], i8[:, 0:2])
                    nc.vector.tensor_copy(meta[:, 2:4], m8[:, 0:2])
                    s01 = spool.tile([128, 1], F32, tag="s01")
                    nc.vector.tensor_add(s01, m8[:, 0:1], m8[:, 1:2])
                    r01 = spool.tile([128, 1], F32, tag="r01")
                    nc.vector.reciprocal(r01, s01)
                    nc.vector.tensor_scalar_mul(meta[:, 4:6], m8[:, 0:2], r01)
                    nc.vector.memset(meta[:, 6:8], 0.0)
                    nc.sync.dma_start(meta_o[row0:row0 + TILE, :], meta)
    return nc


def build_phase2(n_tok=NT):
    """dispatch/combine construction from per-slot codes."""
    ntile = n_tok // TILE
    nc = bass.Bass()
    codes = nc.dram_tensor("codes", [n_tok, 4], F32, kind="ExternalInput")
    iota = nc.dram_tensor("iota", [128, 128], F32, kind="ExternalInput")
    disp_o = nc.dram_tensor("disp", [n_tok, E * K], F32, kind="ExternalOutput")
    comb_o = nc.dram_tensor("comb", [n_tok, E * K], F32, kind="ExternalOutput")

    with SplitDrainTileContext(nc) as tc:
        with (
            tc.tile_pool(name="const", bufs=1) as cpool,
            tc.tile_pool(name="work", bufs=4) as wpool,
        ):
            io_sb = cpool.tile([128, 128], F32)
            nc.sync.dma_start(io_sb, iota[:, :])
            for i in range(ntile):
                row0 = i * TILE
                cd = wpool.tile([128, 4], F32, tag="cd")
                nc.sync.dma_start(cd, codes[row0:row0 + TILE, :])
                d1 = wpool.tile([128, 128], F32, tag="d1")
                nc.vector.tensor_scalar(
                    d1, io_sb, cd[:, 0:1], None, op0=mybir.AluOpType.is_equal)
                dd = wpool.tile([128, 64, 2], F32, tag="dd")
                nc.vector.scalar_tensor_tensor(
                    dd.rearrange("p a b -> p (a b)"), io_sb, cd[:, 1:2], d1,
                    op0=mybir.AluOpType.is_equal, op1=mybir.AluOpType.add)
                cc = wpool.tile([128, 64, 2], F32, tag="cc")
                nc.gpsimd.tensor_scalar_mul(cc[:, :, 0:1], dd[:, :, 0:1], cd[:, 2:3])
                nc.gpsimd.tensor_scalar_mul(cc[:, :, 1:2], dd[:, :, 1:2], cd[:, 3:4])
                nc.sync.dma_start(
                    disp_o[row0:row0 + TILE, :], dd.rearrange("p a b -> p (a b)"))
                nc.sync.dma_start(
                    comb_o[row0:row0 + TILE, :], cc.rearrange("p a b -> p (a b)"))
    return nc


_PROGS = {}
LAST_STATS = {}


def _progs(n_tok=NT):
    if n_tok not in _PROGS:
        _PROGS[n_tok] = (build_phase1(n_tok), build_phase2(n_tok))
    return _PROGS[n_tok]


def _host_prep(x, W):
    xf = np.ascontiguousarray(x.reshape(N, D).T)          # [D, N] f32
    x_hi = xf.astype(np.float16)
    x_lo = (xf - x_hi.astype(np.float32)).astype(np.float16)

    Ws = (W.astype(np.float64) * W_SCALE).astype(np.float32)   # exact *2^10
    W_hi = Ws.astype(np.float16)
    W_lo = (Ws - W_hi.astype(np.float32)).astype(np.float16)
    whl = np.zeros((128, D), np.float16)
    whl2 = np.zeros((128, D), np.float16)
    for c in range(NCHUNK):
        whl[:, c * 128: c * 128 + 64] = W_hi[:, c * 128:(c + 1) * 128].T
        whl[:, c * 128 + 64: c * 128 + 128] = W_lo[:, c * 128:(c + 1) * 128].T
        whl2[:, c * 128 + 64: c * 128 + 128] = W_hi[:, c * 128:(c + 1) * 128].T
    ident = np.eye(128, dtype=np.float32)
    return x_hi, x_lo, whl, whl2, ident


def _capacity_codes(idx, pn):
    """Exact reference capacity logic.  idx [N,2] int, pn [N,2] f32 ->
    codes [N,4] f32 (f0, f1, pn0, pn1) with f = 2e+k or -1 if dropped."""
    flat = idx.reshape(-1)                                 # [(N*2)] k-minor
    oh = np.zeros((flat.shape[0], E), np.int32)
    oh[np.arange(flat.shape[0]), flat] = 1
    prior = np.cumsum(oh, axis=0, dtype=np.int32) - oh
    pos = prior[np.arange(flat.shape[0]), flat]
    keep = pos < CAPACITY
    kpar = np.tile(np.array([0, 1], np.int64), flat.shape[0] // 2)
    f = np.where(keep, 2 * flat + kpar, -1).astype(np.float32)
    codes = np.empty((idx.shape[0], 4), np.float32)
    codes[:, 0:2] = f.reshape(-1, 2)
    codes[:, 2:4] = pn
    return codes


def kernel(x, W):
    x = np.asarray(x, dtype=np.float32)
    W = np.asarray(W, dtype=np.float32)
    nc1, nc2 = _progs()
    x_hi, x_lo, whl, whl2, ident = _host_prep(x, W)

    cores = list(range(N_CORES))
    in1 = []
    for m in cores:
        sl = slice(m * NT, (m + 1) * NT)
        in1.append({
            "xhiT": np.ascontiguousarray(x_hi[:, sl]),
            "xloT": np.ascontiguousarray(x_lo[:, sl]),
            "whl": whl,
            "whl2": whl2,
            "ident": ident,
        })
    trace = os.environ.get("MOE_PROFILE", "0") == "1"
    t0 = time.monotonic()
    r1 = run_bass_kernel_spmd(nc1, in1, core_ids=cores, trace=trace)
    t1 = time.monotonic()
    LAST_STATS["phase1_wall_s"] = t1 - t0
    LAST_STATS["phase1_exec_ns"] = r1.exec_time_ns
    probs = np.concatenate([r1.results[m]["probs"] for m in cores], axis=0)
    meta = np.concatenate([r1.results[m]["meta"] for m in cores], axis=0)

    idx = meta[:, 0:2].astype(np.int64)
    pn = meta[:, 4:6]
    codes = _capacity_codes(idx, pn)

    iota = np.broadcast_to(
        np.arange(128, dtype=np.float32), (128, 128)).copy()
    in2 = [{"codes": np.ascontiguousarray(codes[m * NT:(m + 1) * NT]),
            "iota": iota} for m in cores]
    t2 = time.monotonic()
    r2 = run_bass_kernel_spmd(nc2, in2, core_ids=cores, trace=trace)
    t3 = time.monotonic()
    LAST_STATS["phase2_wall_s"] = t3 - t2
    LAST_STATS["phase2_exec_ns"] = r2.exec_time_ns
    disp = np.concatenate([r2.results[m]["disp"] for m in cores], axis=0)
    comb = np.concatenate([r2.results[m]["comb"] for m in cores], axis=0)

    dispatch = disp.reshape(B, S, E, K)
    combine = comb.reshape(B, S, E, K)
    router_probs = probs.reshape(B, S, E)
    top_k_probs = pn.reshape(B, S, K).astype(np.float32)
    return dispatch, combine, router_probs, top_k_probs


# revision 18
# speedup vs baseline: 2.0289x; 2.0289x over previous
"""MoE router (top-2, capacity=768) on 8 Trainium2 NeuronCores.

Strategy
--------
Data-parallel over tokens: core m owns tokens [m*4096, (m+1)*4096) of the
flattened [32768, 1024] input.

Phase 1 (device, DMA-bound): router logits via fp16 hi/lo-split matmuls
(exact fp32-quality: x = x_hi + x_lo, W' = 1024*W = W_hi + W_lo, keep the
three significant cross terms), softmax (ACT exp with accumulated sum),
top-2 via the DVE max8/max_index instructions.  Emits router_probs and a
small per-token meta record (idx0, idx1, p0, p1, pn0, pn1).

Host exchange (tiny): exact global sequential capacity assignment in flat
(token, k) order from the 32768x2 index table -- the inherently sequential
cross-shard part -- then per-slot target codes f = 2*e + k (or -1 if the
slot is over capacity).

Phase 2 (device, write-bound): builds dispatch/combine [tokens, 64, 2] by
comparing an iota row 0..127 against the per-token codes; combine is
dispatch scaled by the renormalized top-2 probs.
"""

import os
import time

import numpy as np

import concourse.bass as bass
import concourse.mybir as mybir
import bass_rust
from concourse.tile import TileContext
from concourse.vector_clock import ScopedClock
from concourse.bass_utils import run_bass_kernel_spmd


class SplitDrainTileContext(TileContext):
    """TileContext that caps sem-waits at one per instruction -- the
    walrus build in this container rejects any instruction carrying more
    than one sync wait ('Too many sync wait commands').  Excess waits are
    hoisted onto same-engine NoOps emitted just before the instruction,
    which is semantically identical (the engine queue is FIFO)."""

    MAX_WAITS = 1

    def _hoist_excess_waits(self, inst):
        si = inst.sync_info
        if si is None or not si.on_wait or len(si.on_wait) <= self.MAX_WAITS:
            return
        waits = list(si.on_wait)
        keep, extra = waits[: self.MAX_WAITS], waits[self.MAX_WAITS:]
        for w in extra:
            nop = mybir.InstNoOp(
                name=self.nc.get_next_instruction_name(), ins=[], outs=[],
                engine=inst.engine)
            nop.sync_info = bass_rust.SyncInfo(on_wait=[w], on_update=[])
            self._add_instruction(nop)
        si.on_wait = keep

    def _commit_instruction(self, inst, lazy_reg_writes=True):
        if inst.engine != mybir.EngineType.Unassigned:
            self._hoist_excess_waits(inst)
        return super()._commit_instruction(inst, lazy_reg_writes)

    def _drain_and_barrier(self, tick_clock, wait_clock):
        ticks = list(tick_clock.global_clock)
        n = len(ticks)
        for i, t in enumerate(ticks):
            if t:
                d = self.nc.sync.drain()
                clk = bass_rust.VectorClock(
                    [t if j == i else 0 for j in range(n)])
                wait_clock.add_sem_waits(d.ins, ScopedClock({None: clk}))
        self.nc.all_engine_barrier()
        assert self.sems is not None
        popped = self.nc._tile_sem_poison_stack.pop()
        assert popped is self._sem_poison
        self.nc.clear_and_free_semaphores(list(self.sems.allocated().values()))
        self.nc.all_engine_barrier()

F32 = mybir.dt.float32
F16 = mybir.dt.float16
BF16 = mybir.dt.bfloat16
U32 = mybir.dt.uint32

N_CORES = 8
B, S, D, E, K = 4, 8192, 1024, 64, 2
N = B * S                    # 32768 tokens
NT = N // N_CORES            # 4096 tokens per core
CAPACITY = 768
W_SCALE = 1024.0             # router weight pre-scale (undone inside exp)
TILE = 128                   # tokens per tile (partition dim)
GROUP = 512                  # tokens per matmul group (PSUM free dim)
NCHUNK = D // 128            # contraction chunks


def build_phase1(n_tok=NT):
    """logits + softmax + top2.  x inputs are host-prearranged so that each
    512-token group is one contiguous 2D DMA: x*T [ngroup*128, 8*512] with
    row (g*128+p), col (c*512+t) = x[c*128+p (feature), g*512+t (token)].

    Outputs are partition-major: probs [128, ntile*64], i8/m8 [128, ntile*8]
    (top-8 indices / values per token at partition p, block tile).
    """
    ngroup = n_tok // GROUP
    ntile = n_tok // TILE
    nc = bass.Bass()
    # per group g, rows g*128..: cols [0 : 8*512] = x_hi chunks, then x_lo
    xhl = nc.dram_tensor("xhl", [ngroup * 128, 2 * NCHUNK * GROUP], F16,
                         kind="ExternalInput")
    # whl[:, c*128 : c*128+64]  = W_hi[:, c*128:(c+1)*128].T   (fp16)
    # whl[:, c*128+64 : c*128+128] = W_lo chunk transposed
    whl = nc.dram_tensor("whl", [128, D], F16, kind="ExternalInput")
    ident = nc.dram_tensor("ident", [128, 128], F32, kind="ExternalInput")
    probs_o = nc.dram_tensor("probs", [128, ntile * E], F32, kind="ExternalOutput")
    i8_o = nc.dram_tensor("i8", [128, ntile * 8], U32, kind="ExternalOutput")
    m8_o = nc.dram_tensor("m8", [128, ntile * 8], F32, kind="ExternalOutput")

    with SplitDrainTileContext(nc) as tc:
        with (
            tc.tile_pool(name="const", bufs=1) as cpool,
            tc.tile_pool(name="xin", bufs=6) as xpool,
            tc.tile_pool(name="work", bufs=4) as wpool,
            tc.tile_pool(name="small", bufs=12) as spool,
            tc.tile_pool(name="stage", bufs=3) as gpool,
            tc.tile_pool(name="ps_mm", bufs=2, space="PSUM") as pmm,
            tc.tile_pool(name="ps_tr", bufs=3, space="PSUM") as ptr,
            tc.tile_pool(name="ps_wu", bufs=1, space="PSUM") as pwu,
        ):
            w_sb = cpool.tile([128, D], F16)
            nc.sync.dma_start(w_sb, whl[:, :])
            id_sb = cpool.tile([128, 128], F32)
            nc.sync.dma_start(id_sb, ident[:, :])
            i8_all = cpool.tile([128, ntile * 8], U32)
            m8_all = cpool.tile([128, ntile * 8], F32)

            # HAM warm-up: ~4us of dummy matmuls so the PE clock-gate opens
            # while the first x tiles are still in flight on DMA.
            wu = pwu.tile([128, GROUP], F32)
            for r in range(10):
                nc.tensor.matmul(wu, lhsT=w_sb[:, 0:128], rhs=w_sb[:, 0:GROUP],
                                 start=True, stop=True, skip_group_check=True)

            def load_group(g):
                xb = xpool.tile([128, 2, NCHUNK, GROUP], F16, tag="xb")
                nc.sync.dma_start(
                    xb, xhl[g * 128:(g + 1) * 128, :]
                    .rearrange("p (h c t) -> p h c t", h=2, t=GROUP))
                return xb[:, 0], xb[:, 1]

            def logits_tail(g, ps):
                """From accumulated logits^T psum to probs/top2 for group g."""
                lo_sb = wpool.tile([64, GROUP], F32, tag="lo_sb")
                nc.scalar.copy(lo_sb, ps[64:128, :])
                logT = wpool.tile([64, GROUP], F32, tag="logT")
                nc.vector.tensor_add(logT, ps[0:64, :], lo_sb)

                pT = ptr.tile([128, 4 * E], F32, tag="pT")
                for j in range(4):
                    nc.tensor.transpose(
                        pT[:, j * E:(j + 1) * E],
                        logT[:, j * 128:(j + 1) * 128],
                        id_sb[0:64, 0:64],
                    )
                probs_g = gpool.tile([128, 4 * E], F32, tag="probs_g")
                for j in range(4):
                    i = g * 4 + j          # global tile index
                    ex = wpool.tile([128, E], F32, tag="ex")
                    sumex = spool.tile([128, 1], F32, tag="sumex")
                    nc.scalar.activation(
                        ex, pT[:, j * E:(j + 1) * E],
                        mybir.ActivationFunctionType.Exp,
                        scale=1.0 / W_SCALE, accum_out=sumex,
                    )
                    rs = spool.tile([128, 1], F32, tag="rs")
                    nc.vector.reciprocal(rs, sumex)
                    nc.vector.tensor_scalar_mul(
                        probs_g[:, j * E:(j + 1) * E], ex, rs)
                    m8 = m8_all[:, i * 8:(i + 1) * 8]
                    nc.vector.max(m8, ex)
                    nc.vector.max_index(i8_all[:, i * 8:(i + 1) * 8], m8, ex)
                nc.sync.dma_start(
                    probs_o[:, g * 4 * E:(g + 1) * 4 * E], probs_g)

            for q in range(ngroup // 2):
                g0, g1 = 2 * q, 2 * q + 1
                xh0, xl0 = load_group(g0)
                xh1, xl1 = load_group(g1)
                ps0 = pmm.tile([128, GROUP], F32, tag="ps0")
                ps1 = pmm.tile([128, GROUP], F32, tag="ps1")
                for c in range(NCHUNK):
                    nc.tensor.matmul(
                        ps0, lhsT=w_sb[:, c * 128:(c + 1) * 128],
                        rhs=xh0[:, c, :], start=(c == 0), stop=False,
                        skip_group_check=True)
                for c in range(NCHUNK):
                    nc.tensor.matmul(
                        ps1, lhsT=w_sb[:, c * 128:(c + 1) * 128],
                        rhs=xh1[:, c, :], start=(c == 0), stop=False,
                        skip_group_check=True)
                # x_lo * W_hi for both groups concurrently on disjoint
                # column groups of the PE array (adds into rows 0:64 of
                # ps0 and rows 64:128 of ps1 -- both are summed later).
                for c in range(NCHUNK):
                    nc.tensor.matmul(
                        ps0[0:64, :], lhsT=w_sb[:, c * 128: c * 128 + 64],
                        rhs=xl0[:, c, :], start=False,
                        stop=(c == NCHUNK - 1), tile_position=(0, 0),
                        skip_group_check=True)
                    nc.tensor.matmul(
                        ps1[64:128, :], lhsT=w_sb[:, c * 128: c * 128 + 64],
                        rhs=xl1[:, c, :], start=False,
                        stop=(c == NCHUNK - 1), tile_position=(0, 64),
                        skip_group_check=True)
                logits_tail(g0, ps0)
                logits_tail(g1, ps1)
                c0, c1 = g0 * 4 * 8, (g1 + 1) * 4 * 8
                nc.sync.dma_start(i8_o[:, c0:c1], i8_all[:, c0:c1])
                nc.sync.dma_start(m8_o[:, c0:c1], m8_all[:, c0:c1])
    return nc


def build_phase2(n_tok=NT):
    """dispatch/combine from per-slot codes (partition-major layouts).

    combine = (iota==f0)*pn0 + (iota==f1)*pn1   (two fused DVE ops + add)
    dispatch = Sign(combine)                    (ACT; pn > 0 strictly)
    """
    ntile = n_tok // TILE
    ngroup = n_tok // GROUP
    nc = bass.Bass()
    codes = nc.dram_tensor("codes", [128, ntile * 4], F32, kind="ExternalInput")
    iota = nc.dram_tensor("iota", [128, 128], F32, kind="ExternalInput")
    disp_o = nc.dram_tensor("disp", [128, ntile * E * K], F32, kind="ExternalOutput")
    comb_o = nc.dram_tensor("comb", [128, ntile * E * K], F32, kind="ExternalOutput")

    with SplitDrainTileContext(nc) as tc:
        with (
            tc.tile_pool(name="const", bufs=1) as cpool,
            tc.tile_pool(name="work", bufs=8) as wpool,
            tc.tile_pool(name="stage", bufs=4) as gpool,
        ):
            io_sb = cpool.tile([128, 128], F32)
            nc.sync.dma_start(io_sb, iota[:, :])
            io_bf = cpool.tile([128, 128], BF16)
            nc.vector.tensor_copy(io_bf, io_sb)
            codes_sb = cpool.tile([128, ntile * 4], F32)
            nc.sync.dma_start(codes_sb, codes[:, :])
            for g in range(ngroup):
                dd_g = gpool.tile([128, 4 * 128], F32, tag="dd")
                cc_g = gpool.tile([128, 4 * 128], F32, tag="cc")
                for j in range(4):
                    i = g * 4 + j
                    cd0 = codes_sb[:, i * 4: i * 4 + 1]
                    cd1 = codes_sb[:, i * 4 + 1: i * 4 + 2]
                    pn0 = codes_sb[:, i * 4 + 2: i * 4 + 3]
                    pn1 = codes_sb[:, i * 4 + 3: i * 4 + 4]
                    e1 = wpool.tile([128, 128], F32, tag="e1")
                    nc.vector.tensor_scalar(
                        e1, io_sb, cd0, pn0,
                        op0=mybir.AluOpType.is_equal, op1=mybir.AluOpType.mult)
                    e2 = wpool.tile([128, 128], F32, tag="e2")
                    nc.vector.tensor_scalar(
                        e2, io_sb, cd1, pn1,
                        op0=mybir.AluOpType.is_equal, op1=mybir.AluOpType.mult)
                    cc = cc_g[:, j * 128:(j + 1) * 128]
                    nc.gpsimd.tensor_tensor(cc, e1, e2, op=mybir.AluOpType.add)
                    nc.scalar.sign(dd_g[:, j * 128:(j + 1) * 128], cc)
                nc.sync.dma_start(
                    disp_o[:, g * 512:(g + 1) * 512], dd_g)
                nc.scalar.dma_start(
                    comb_o[:, g * 512:(g + 1) * 512], cc_g)
    return nc


_PROGS = {}
LAST_STATS = {}


def _progs(n_tok=NT):
    if n_tok not in _PROGS:
        _PROGS[n_tok] = (build_phase1(n_tok), build_phase2(n_tok))
    return _PROGS[n_tok]


def _host_prep(x, W):
    xf = np.ascontiguousarray(x.reshape(N, D).T)          # [D, N] f32
    x_hi = xf.astype(np.float16)
    x_lo = (xf - x_hi.astype(np.float32)).astype(np.float16)

    Ws = (W.astype(np.float64) * W_SCALE).astype(np.float32)   # exact *2^10
    W_hi = Ws.astype(np.float16)
    W_lo = (Ws - W_hi.astype(np.float32)).astype(np.float16)
    whl = np.zeros((128, D), np.float16)
    for c in range(NCHUNK):
        whl[:, c * 128: c * 128 + 64] = W_hi[:, c * 128:(c + 1) * 128].T
        whl[:, c * 128 + 64: c * 128 + 128] = W_lo[:, c * 128:(c + 1) * 128].T
    ident = np.eye(128, dtype=np.float32)
    return x_hi, x_lo, whl, ident


def _x_group_layout(xt_core):
    """[1024 feat, n_tok] -> [ngroup*128, 8*512] group-contiguous rows."""
    n_tok = xt_core.shape[1]
    ngroup = n_tok // GROUP
    a = xt_core.reshape(NCHUNK, 128, ngroup, GROUP)       # [c, p, g, t]
    return np.ascontiguousarray(
        a.transpose(2, 1, 0, 3)).reshape(ngroup * 128, NCHUNK * GROUP)


def _capacity_codes(idx, pn):
    """Exact reference capacity logic.  idx [N,2] int, pn [N,2] f32 ->
    codes [N,4] f32 (f0, f1, pn0, pn1) with f = 2e+k or -1 if dropped."""
    flat = idx.reshape(-1)                                 # [(N*2)] k-minor
    oh = np.zeros((flat.shape[0], E), np.int32)
    oh[np.arange(flat.shape[0]), flat] = 1
    prior = np.cumsum(oh, axis=0, dtype=np.int32) - oh
    pos = prior[np.arange(flat.shape[0]), flat]
    keep = pos < CAPACITY
    kpar = np.tile(np.array([0, 1], np.int64), flat.shape[0] // 2)
    f = np.where(keep, 2 * flat + kpar, -1).astype(np.float32)
    codes = np.empty((idx.shape[0], 4), np.float32)
    codes[:, 0:2] = f.reshape(-1, 2)
    codes[:, 2:4] = pn
    return codes


def _pmaj_to_tokens(a, cols):
    """[128, ntile*cols] partition-major -> [ntile*128, cols] token-major."""
    ntile = a.shape[1] // cols
    return np.ascontiguousarray(
        a.reshape(128, ntile, cols).transpose(1, 0, 2)).reshape(-1, cols)


def _tokens_to_pmaj(a, cols):
    """[ntile*128, cols] token-major -> [128, ntile*cols] partition-major."""
    ntile = a.shape[0] // 128
    return np.ascontiguousarray(
        a.reshape(ntile, 128, cols).transpose(1, 0, 2)).reshape(128, -1)


def kernel(x, W):
    x = np.asarray(x, dtype=np.float32)
    W = np.asarray(W, dtype=np.float32)
    nc1, nc2 = _progs()
    x_hi, x_lo, whl, ident = _host_prep(x, W)

    cores = list(range(N_CORES))
    in1 = []
    for m in cores:
        sl = slice(m * NT, (m + 1) * NT)
        hi = _x_group_layout(x_hi[:, sl])
        lo = _x_group_layout(x_lo[:, sl])
        in1.append({
            "xhl": np.concatenate([hi, lo], axis=1),
            "whl": whl,
            "ident": ident,
        })
    trace = os.environ.get("MOE_PROFILE", "0") == "1"
    t0 = time.monotonic()
    r1 = run_bass_kernel_spmd(nc1, in1, core_ids=cores, trace=trace)
    t1 = time.monotonic()
    LAST_STATS["phase1_wall_s"] = t1 - t0
    LAST_STATS["phase1_exec_ns"] = r1.exec_time_ns
    LAST_STATS["phase1_profile_json"] = r1.profile_json
    LAST_STATS["phase1_trace"] = r1.instructions_and_trace
    probs = np.concatenate(
        [_pmaj_to_tokens(r1.results[m]["probs"], E) for m in cores], axis=0)
    i8 = np.concatenate(
        [_pmaj_to_tokens(r1.results[m]["i8"], 8) for m in cores], axis=0)
    m8 = np.concatenate(
        [_pmaj_to_tokens(r1.results[m]["m8"], 8) for m in cores], axis=0)

    idx = i8[:, 0:2].astype(np.int64)
    p01 = m8[:, 0:2]
    pn = p01 / p01.sum(axis=1, keepdims=True)
    codes = _capacity_codes(idx, pn)

    iota = np.broadcast_to(
        np.arange(128, dtype=np.float32), (128, 128)).copy()
    in2 = [{"codes": _tokens_to_pmaj(codes[m * NT:(m + 1) * NT], 4),
            "iota": iota} for m in cores]
    t2 = time.monotonic()
    r2 = run_bass_kernel_spmd(nc2, in2, core_ids=cores, trace=trace)
    t3 = time.monotonic()
    LAST_STATS["phase2_wall_s"] = t3 - t2
    LAST_STATS["phase2_exec_ns"] = r2.exec_time_ns
    LAST_STATS["phase2_profile_json"] = r2.profile_json
    LAST_STATS["phase2_trace"] = r2.instructions_and_trace
    disp = np.concatenate(
        [_pmaj_to_tokens(r2.results[m]["disp"], E * K) for m in cores], axis=0)
    comb = np.concatenate(
        [_pmaj_to_tokens(r2.results[m]["comb"], E * K) for m in cores], axis=0)

    dispatch = disp.reshape(B, S, E, K)
    combine = comb.reshape(B, S, E, K)
    router_probs = probs.reshape(B, S, E)
    top_k_probs = pn.reshape(B, S, K).astype(np.float32)
    return dispatch, combine, router_probs, top_k_probs

